# revision 1
# baseline (speedup 1.0000x reference)
"""CrossAttnBlock TRN2 kernel: 8-way (batch x l-half) sharded, collective-free.

Reference math (b=4, c=64, h=64, w=32, dim=256, HEADS=8, l=h*w=2048):
  zf = z.reshape(b, dim, l).T            # [b, l, dim]
  q  = x.reshape(b, c, l).T              # [b, l, c]
  k  = (zf @ Wk + bk) -> [b, H, l, c];  v likewise
  S  = q @ k.T / sqrt(c); A = softmax(S, -1); P = A @ v
  out = (P heads-concat) @ Wo + bo       # [b, l, c]
  return x + out.reshape(b, c, h, w)     # raw-memory reinterpretation

Per-core (core = bi*2 + half): full K/V projection for batch bi, attention +
out-proj for l rows [half*1024, (half+1)*1024). All layouts fall out of raw
input memory: z raw = zf^T ([dim, l]), x raw = q^T ([c, l]), out rows = raw
flat output. Scores are computed transposed (S^T [m, l]) so the AV contraction
runs with m on partitions; softmax denominators come from a ones-augmented V
column; normalization is applied per-head to the [l, c]-layout out-proj
partials where the divisor is a per-partition scalar.
"""
import ml_dtypes
import numpy as np

import concourse.bass as bass
import concourse.mybir as mybir
import concourse.tile as tile
from concourse import bacc
from concourse.bass_utils import run_bass_kernel_spmd
from concourse.masks import make_identity

F32 = mybir.dt.float32
F32R = mybir.dt.float32r
BF16 = mybir.dt.bfloat16

B, C, H, W = 4, 64, 64, 32
DIM = 256
HEADS = 8
L = H * W            # 2048
LH = L // 2          # 1024 per core
INNER = HEADS * C    # 512
N_CORES = 8

_CACHE = {}


def _r(ap):
    return ap.bitcast(F32R) if ap.dtype == F32 else ap


def build_nc():
    nc = bacc.Bacc("TRN2", target_bir_lowering=False, debug=False,
                   num_devices=N_CORES)
    xq = nc.dram_tensor("xq", [C, LH], BF16, kind="ExternalInput")
    xr = nc.dram_tensor("xr", [128, LH // 128, C], F32, kind="ExternalInput")
    zb = nc.dram_tensor("zb", [DIM, L], BF16, kind="ExternalInput")
    Wk = nc.dram_tensor("Wk", [DIM, INNER], BF16, kind="ExternalInput")
    Wv = nc.dram_tensor("Wv", [DIM, INNER], BF16, kind="ExternalInput")
    Wo = nc.dram_tensor("Wo", [C, HEADS, C], BF16, kind="ExternalInput")
    bk = nc.dram_tensor("bk", [128, 4], F32, kind="ExternalInput")
    bv = nc.dram_tensor("bv", [1, INNER], BF16, kind="ExternalInput")
    bo = nc.dram_tensor("bo", [1, C], BF16, kind="ExternalInput")
    ones_b = nc.dram_tensor("ones_b", [128, 128], BF16, kind="ExternalInput")
    OUT = nc.dram_tensor("out", [LH, C], F32, kind="ExternalOutput")

    NMT = L // 128       # 16 m-tiles
    NLS = LH // 128      # 8 l-subtiles

    with tile.TileContext(nc) as tc:
        with (
            tc.tile_pool(name="const", bufs=1) as cp,
            tc.tile_pool(name="pexp", bufs=3) as pe_pool,
            tc.tile_pool(name="small", bufs=3) as sp,
            tc.tile_pool(name="ps_proj", bufs=2, space="PSUM") as ps_proj,
            tc.tile_pool(name="ps_s", bufs=2, space="PSUM") as ps_s,
            tc.tile_pool(name="ps_pt", bufs=1, space="PSUM") as ps_pt,
        ):
            # ---- constants / inputs in SBUF ----
            z_sb = [cp.tile([128, L], BF16, tag=f"z{d}", name=f"z{d}") for d in range(2)]
            for d in range(2):
                nc.sync.dma_start(out=z_sb[d], in_=zb[d * 128:(d + 1) * 128, :])
            wk_sb = [cp.tile([128, INNER], BF16, tag=f"wk{d}", name=f"wk{d}") for d in range(2)]
            wv_sb = [cp.tile([128, INNER], BF16, tag=f"wv{d}", name=f"wv{d}") for d in range(2)]
            for d in range(2):
                nc.sync.dma_start(out=wk_sb[d], in_=Wk[d * 128:(d + 1) * 128, :])
                nc.sync.dma_start(out=wv_sb[d], in_=Wv[d * 128:(d + 1) * 128, :])
            wo_sb = cp.tile([C, HEADS, C], BF16, tag="wo")
            nc.sync.dma_start(out=wo_sb, in_=Wo[:, :, :])
            x_sb = cp.tile([128, LH], BF16, tag="x")
            nc.sync.dma_start(out=x_sb[0:C, :], in_=xq[:, :])
            nc.sync.dma_start(out=x_sb[C:2 * C, :], in_=xq[:, :])
            xr_sb = cp.tile([128, NLS, C], F32, tag="xr")
            nc.sync.dma_start(out=xr_sb, in_=xr[:, :, :])
            bk_sb = cp.tile([128, 4], F32, tag="bk")
            nc.sync.dma_start(out=bk_sb, in_=bk[:, :])
            bv_sb = cp.tile([1, INNER], BF16, tag="bv")
            nc.sync.dma_start(out=bv_sb, in_=bv[:, :])
            bo_sb = cp.tile([1, C], BF16, tag="bo")
            nc.sync.dma_start(out=bo_sb, in_=bo[:, :])
            ones_bf = cp.tile([1, 128], BF16, tag="ones_bf")
            nc.sync.dma_start(out=ones_bf, in_=ones_b[0:1, :])
            ident = cp.tile([8, 8], BF16, tag="ident")
            make_identity(nc, ident)

            kT_sb = [cp.tile([128, L], BF16, tag=f"kT{t}", name=f"kT{t}") for t in range(4)]
            v_sb = cp.tile([128, NMT, HEADS, C + 1], BF16, tag="v")
            nc.sync.dma_start(
                out=v_sb[:, :, :, C:C + 1],
                in_=ones_b.rearrange("p (a b c) -> p a b c", a=NMT, b=HEADS))
            pt_sb = [cp.tile([C + 1, LH], BF16, tag=f"pt{h}", name=f"pt{h}") for h in range(HEADS)]
            sums_sb = cp.tile([HEADS, LH], BF16, tag="sums")

            # ---- Phase A: kT[ci, m] = (Wk^T @ zf^T) + bk ----
            for t in range(4):
                for s in range(4):        # m slice (512 wide)
                    pk = ps_proj.tile([128, 512], F32, tag="proj")
                    for d in range(2):
                        nc.tensor.matmul(
                            pk,
                            wk_sb[d][:, t * 128:(t + 1) * 128],
                            z_sb[d][:, s * 512:(s + 1) * 512],
                            start=(d == 0), stop=(d == 1))
                    nc.vector.tensor_scalar(
                        out=kT_sb[t][:, s * 512:(s + 1) * 512], in0=pk,
                        scalar1=bk_sb[:, t:t + 1], scalar2=None,
                        op0=mybir.AluOpType.add)
            # ---- Phase B: v[m, ci] = zf @ Wv + bv (ones col appended) ----
            for s in range(NMT):          # m tile (128 rows)
                pv = ps_proj.tile([128, 512], F32, tag="proj")
                for d in range(2):
                    nc.tensor.matmul(
                        pv,
                        z_sb[d][:, s * 128:(s + 1) * 128],
                        wv_sb[d],
                        start=(d == 0), stop=False)
                nc.tensor.matmul(pv, ones_bf, bv_sb, start=False, stop=True)
                nc.vector.tensor_copy(
                    out=v_sb[:, s, :, 0:C],
                    in_=pv.rearrange("p (h c) -> p h c", h=HEADS))

            # ---- Phase C: attention per head ----
            for h in range(HEADS):
                t, roff = h // 2, 64 * (h % 2)
                ptp = ps_pt.tile([C + 1, LH], F32, tag="pt")
                for mt in range(NMT):
                    pss = ps_s.tile([128, LH], F32, tag="s")
                    for lh_ in range(2):
                        nc.tensor.matmul(
                            pss[:, lh_ * 512:(lh_ + 1) * 512],
                            kT_sb[t][roff:roff + 64, mt * 128:(mt + 1) * 128],
                            x_sb[roff:roff + C, lh_ * 512:(lh_ + 1) * 512],
                            start=True, stop=True)
                    es = pe_pool.tile([128, LH], BF16, tag="es")
                    nc.scalar.activation(out=es, in_=pss,
                                         func=mybir.ActivationFunctionType.Exp,
                                         scale=float(C) ** -0.5)
                    for lh_ in range(2):
                        nc.tensor.matmul(
                            ptp[:, lh_ * 512:(lh_ + 1) * 512],
                            v_sb[:, mt, h, :],
                            es[:, lh_ * 512:(lh_ + 1) * 512],
                            start=(mt == 0), stop=(mt == NMT - 1))
                nc.vector.tensor_copy(out=pt_sb[h], in_=ptp)
                nc.sync.dma_start(out=sums_sb[h:h + 1, :],
                                  in_=pt_sb[h][C:C + 1, :])

            # ---- Phase D: out-proj + normalize + residual per l-subtile ----
            for ls in range(NLS):
                ptr = ps_proj.tile([128, 8], BF16, tag="proj")
                nc.tensor.transpose(ptr, sums_sb[:, ls * 128:(ls + 1) * 128], ident)
                recip = sp.tile([128, 8], F32, tag="recip")
                nc.vector.reciprocal(out=recip, in_=ptr)
                acc = None
                for h in range(HEADS):
                    po = ps_proj.tile([128, C], F32, tag="proj")
                    nc.tensor.matmul(
                        po,
                        pt_sb[h][0:C, ls * 128:(ls + 1) * 128],
                        wo_sb[:, h, :],
                        start=True, stop=(h != 0))
                    if h == 0:
                        nc.tensor.matmul(po, ones_bf, bo_sb,
                                         start=False, stop=True)
                    tmp = sp.tile([128, C], F32, tag="tmp")
                    nc.vector.tensor_scalar(
                        out=tmp, in0=po, scalar1=recip[:, h:h + 1],
                        scalar2=None, op0=mybir.AluOpType.mult)
                    if h == 0:
                        acc = sp.tile([128, C], F32, tag="oacc")
                        nc.vector.tensor_tensor(
                            out=acc, in0=xr_sb[:, ls, :], in1=tmp,
                            op=mybir.AluOpType.add)
                    else:
                        nc.vector.tensor_tensor(
                            out=acc, in0=acc, in1=tmp,
                            op=mybir.AluOpType.add)
                nc.sync.dma_start(out=OUT[ls * 128:(ls + 1) * 128, :], in_=acc)

    nc.compile()
    return nc


def kernel(x, z, Wk, bk, Wv, bv, Wo, bo):
    x = np.ascontiguousarray(x, dtype=np.float32)
    z = np.ascontiguousarray(z, dtype=np.float32)
    if "nc" not in _CACHE:
        _CACHE["nc"] = build_nc()
    nc = _CACHE["nc"]
    shared = {
        "Wk": np.ascontiguousarray(np.asarray(Wk, np.float32).astype(ml_dtypes.bfloat16)),
        "Wv": np.ascontiguousarray(np.asarray(Wv, np.float32).astype(ml_dtypes.bfloat16)),
        "Wo": np.ascontiguousarray(np.asarray(Wo, np.float32)
                                   .reshape(HEADS, C, C).transpose(1, 0, 2)
                                   .astype(ml_dtypes.bfloat16)),
        "bk": np.ascontiguousarray(
            np.asarray(bk, np.float32).reshape(4, 128).T),
        "bv": np.ascontiguousarray(
            np.asarray(bv, np.float32).reshape(1, INNER).astype(ml_dtypes.bfloat16)),
        "bo": np.ascontiguousarray(
            np.asarray(bo, np.float32).reshape(1, C).astype(ml_dtypes.bfloat16)),
        "ones_b": np.ones((128, 128), ml_dtypes.bfloat16),
    }
    in_maps = []
    for core in range(N_CORES):
        bi, half = core // 2, core % 2
        xi = x[bi].reshape(C, L)
        in_maps.append({
            "xq": np.ascontiguousarray(
                xi[:, half * LH:(half + 1) * LH].astype(ml_dtypes.bfloat16)),
            "xr": np.ascontiguousarray(
                x[bi].reshape(-1)[half * LH * C:(half + 1) * LH * C]
                .reshape(LH // 128, 128, C).transpose(1, 0, 2)),
            "zb": np.ascontiguousarray(
                z[bi].reshape(DIM, L).astype(ml_dtypes.bfloat16)),
            **shared,
        })
    _CACHE["in_maps"] = in_maps
    res = run_bass_kernel_spmd(nc, in_maps, list(range(N_CORES)))
    full = np.empty((B, L * C), dtype=np.float32)
    for core in range(N_CORES):
        bi, half = core // 2, core % 2
        full[bi, half * LH * C:(half + 1) * LH * C] = \
            res.results[core]["out"].reshape(-1)
    return full.reshape(B, C, H, W)



# revision 5
# speedup vs baseline: 1.3410x; 1.3410x over previous
"""CrossAttnBlock TRN2 kernel: 8-way (batch x l-half) sharded, collective-free.

Reference math (b=4, c=64, h=64, w=32, dim=256, HEADS=8, l=h*w=2048):
  zf = z.reshape(b, dim, l).T            # [b, l, dim]
  q  = x.reshape(b, c, l).T              # [b, l, c]
  k  = (zf @ Wk + bk) -> [b, H, l, c];  v likewise
  S  = q @ k.T / sqrt(c); A = softmax(S, -1); P = A @ v
  out = (P heads-concat) @ Wo + bo       # [b, l, c]
  return x + out.reshape(b, c, h, w)     # raw-memory reinterpretation

Per-core (core = bi*2 + half): full K/V projection for batch bi, attention +
out-proj for l rows [half*1024, (half+1)*1024).

Phase C processes head PAIRS with row-tiled score matmuls (heads 2p/2p+1 in
partition halves of kT, running concurrently in the PE array), and splits the
softmax exp between the Scalar engine (table exp) and the Vector engine
(Schraudolph bit-trick exp: bf16 bits = int16(s*A + B)). Softmax denominators
come from a ones-augmented V column; per-head normalization + out-proj +
residual run as an interleaved phase D (out-proj on PE, scale on DVE, head-sum
tree on GpSimd). bo and bv fold into the host-side residual (out-proj is
linear: P@Wo + (bv*denom)@Wo -> bv@Wo constant after normalize); bk is added
on the kT drain path.
"""
import ml_dtypes
import numpy as np

import concourse.bass as bass
import concourse.mybir as mybir
import concourse.tile as tile
from concourse import bacc
from concourse.bass_utils import run_bass_kernel_spmd
from concourse.masks import make_identity

F32 = mybir.dt.float32
BF16 = mybir.dt.bfloat16
I16 = mybir.dt.int16

B, C, H, W = 4, 64, 64, 32
DIM = 256
HEADS = 8
L = H * W            # 2048
LH = L // 2          # 1024 per core
INNER = HEADS * C    # 512
N_CORES = 8
NMT = L // 128       # 16 m-tiles
NLS = LH // 128      # 8 l-subtiles

SCALE = float(C) ** -0.5
# Schraudolph exp in bf16 bits: bf16(exp(s*SCALE)) ~ int16(s*EXP_A + EXP_B)
EXP_A = (2.0 ** 7 / float(np.log(2.0))) * SCALE
EXP_B = 16256.0 - 7.4

_CACHE = {}


def build_nc():
    nc = bacc.Bacc("TRN2", target_bir_lowering=False, debug=False,
                   num_devices=N_CORES)
    zb = nc.dram_tensor("zb", [DIM, L], BF16, kind="ExternalInput")
    Wk = nc.dram_tensor("Wk", [DIM, INNER], BF16, kind="ExternalInput")
    Wv = nc.dram_tensor("Wv", [DIM, INNER], BF16, kind="ExternalInput")
    xq = nc.dram_tensor("xq", [C, LH], BF16, kind="ExternalInput")
    xr = nc.dram_tensor("xr", [128, NLS, C], F32, kind="ExternalInput")
    Wo = nc.dram_tensor("Wo", [C, HEADS, C], BF16, kind="ExternalInput")
    bk = nc.dram_tensor("bk", [128, 4], F32, kind="ExternalInput")
    ones_b = nc.dram_tensor("ones_b", [128, 128], BF16, kind="ExternalInput")
    OUT = nc.dram_tensor("out", [LH, C], F32, kind="ExternalOutput")

    AF = mybir.ActivationFunctionType
    OP = mybir.AluOpType

    with tile.TileContext(nc) as tc:
        with (
            tc.tile_pool(name="const", bufs=1) as cp,
            tc.tile_pool(name="es", bufs=4) as ep,
            tc.tile_pool(name="sm", bufs=3) as sp,
            tc.tile_pool(name="ps_s", bufs=4, space="PSUM") as ps_s,
            tc.tile_pool(name="ps_pt", bufs=4, space="PSUM") as ps_pt,
        ):
            # ---- inputs to SBUF (z + Wk first so phase A starts early) ----
            z_sb = cp.tile([128, 2, L], BF16, tag="z")
            for d in range(2):
                nc.sync.dma_start(out=z_sb[:, d], in_=zb[d * 128:(d + 1) * 128, :])
            wk_sb = cp.tile([128, 2, INNER], BF16, tag="wk")
            for d in range(2):
                nc.sync.dma_start(out=wk_sb[:, d], in_=Wk[d * 128:(d + 1) * 128, :])
            bk_sb = cp.tile([128, 4], F32, tag="bk")
            nc.sync.dma_start(out=bk_sb, in_=bk[:, :])
            wv_sb = cp.tile([128, 2, INNER], BF16, tag="wv")
            for d in range(2):
                nc.sync.dma_start(out=wv_sb[:, d], in_=Wv[d * 128:(d + 1) * 128, :])
            x_sb = cp.tile([128, LH], BF16, tag="x")
            nc.sync.dma_start(out=x_sb[0:C, :], in_=xq[:, :])
            nc.sync.dma_start(out=x_sb[C:2 * C, :], in_=xq[:, :])
            xr_sb = cp.tile([128, NLS, C], F32, tag="xr")
            nc.sync.dma_start(out=xr_sb, in_=xr[:, :, :])
            wo_sb = cp.tile([C, HEADS, C], BF16, tag="wo")
            nc.sync.dma_start(out=wo_sb, in_=Wo[:, :, :])
            ident = cp.tile([8, 8], BF16, tag="ident")
            make_identity(nc, ident)

            kT = [cp.tile([128, L], BF16, tag=f"kT{t}", name=f"kT{t}")
                  for t in range(4)]
            v_sb = cp.tile([128, NMT, HEADS, C + 1], BF16, tag="v")
            nc.sync.dma_start(
                out=v_sb[:, :, :, C:C + 1],
                in_=ones_b.rearrange("p (a b c) -> p a b c", a=NMT, b=HEADS))
            pt_sb = [cp.tile([C + 1, LH], BF16, tag=f"pt{h}", name=f"pt{h}")
                     for h in range(HEADS)]
            sums_sb = cp.tile([HEADS, LH], BF16, tag="sums")

            # ---- Phase A: kT[ci, m] = (Wk^T @ zf^T) + bk ----
            for t in range(4):
                for s in range(4):
                    pk = ps_s.tile([128, 512], F32, tag="s", name="pk")
                    for d in range(2):
                        nc.tensor.matmul(
                            pk,
                            wk_sb[:, d, t * 128:(t + 1) * 128],
                            z_sb[:, d, s * 512:(s + 1) * 512],
                            start=(d == 0), stop=(d == 1))
                    dst = kT[t][:, s * 512:(s + 1) * 512]
                    if (t * 4 + s) % 2 == 0:
                        nc.vector.tensor_scalar(
                            out=dst, in0=pk, scalar1=bk_sb[:, t:t + 1],
                            scalar2=None, op0=OP.add)
                    else:
                        nc.scalar.activation(
                            out=dst, in_=pk, func=AF.Identity,
                            bias=bk_sb[:, t:t + 1], scale=1.0)

            # ---- Phase B: v[m, h, ci] = zf @ Wv (ones col appended) ----
            for s in range(NMT):
                pv = ps_s.tile([128, 512], F32, tag="s", name="pv")
                for d in range(2):
                    nc.tensor.matmul(
                        pv,
                        z_sb[:, d, s * 128:(s + 1) * 128],
                        wv_sb[:, d],
                        start=(d == 0), stop=(d == 1))
                pvr = pv.rearrange("p (h c) -> p h c", h=HEADS)
                if s % 2 == 0:
                    nc.vector.tensor_copy(out=v_sb[:, s, :, 0:C], in_=pvr)
                else:
                    nc.scalar.activation(out=v_sb[:, s, :, 0:C], in_=pvr,
                                         func=AF.Copy)

            # ---- Phase D (emitted per l-subtile; interleaved into phase C) ----
            def phase_d(ls):
                lo = ls * 128
                ptr = ps_s.tile([128, 8], BF16, tag="s", name="ptr")
                nc.tensor.transpose(ptr, sums_sb[:, lo:lo + 128], ident)
                recip = sp.tile([128, 8], F32, tag="recip", name="recip")
                nc.vector.reciprocal(out=recip, in_=ptr)
                po = ps_s.tile([128, HEADS, C], F32, tag="s", name="po")
                for h in range(HEADS):
                    nc.tensor.matmul(
                        po[:, h, :],
                        pt_sb[h][0:C, lo:lo + 128],
                        wo_sb[:, h, :],
                        start=True, stop=True)
                tmp = sp.tile([128, HEADS, C], F32, tag="tmp", name="tmp")
                rb = recip.rearrange("p (h o) -> p h o", o=1)
                nc.vector.tensor_tensor(
                    out=tmp, in0=po, in1=rb.broadcast_to([128, HEADS, C]),
                    op=OP.mult)
                t1 = sp.tile([128, 4, C], F32, tag="t1", name="t1")
                nc.gpsimd.tensor_tensor(
                    out=t1, in0=tmp[:, 0:4, :], in1=tmp[:, 4:8, :], op=OP.add)
                t2 = sp.tile([128, 2, C], F32, tag="t2", name="t2")
                nc.gpsimd.tensor_tensor(
                    out=t2, in0=t1[:, 0:2, :], in1=t1[:, 2:4, :], op=OP.add)
                fin = sp.tile([128, C], F32, tag="fin", name="fin")
                nc.gpsimd.tensor_tensor(
                    out=fin, in0=t2[:, 0, :], in1=t2[:, 1, :], op=OP.add)
                nc.gpsimd.tensor_tensor(
                    out=fin, in0=fin, in1=xr_sb[:, ls, :], op=OP.add)
                nc.sync.dma_start(out=OUT[lo:lo + 128, :], in_=fin)

            # ---- Phase C: attention, head pairs row-tiled in the PE array ----
            it = 0
            for lh in range(2):
                lo = lh * 512
                for p in range(4):
                    he, ho = 2 * p, 2 * p + 1
                    ptp_e = ps_pt.tile([C + 1, 512], F32, tag="pt", name="ptpe")
                    ptp_o = ps_pt.tile([C + 1, 512], F32, tag="pt", name="ptpo")
                    for mt in range(NMT):
                        ms = mt * 128
                        se = ps_s.tile([128, 512], F32, tag="s", name="se")
                        so = ps_s.tile([128, 512], F32, tag="s", name="so")
                        nc.tensor.matmul(se, kT[p][0:64, ms:ms + 128],
                                         x_sb[0:64, lo:lo + 512],
                                         start=True, stop=True)
                        nc.tensor.matmul(so, kT[p][64:128, ms:ms + 128],
                                         x_sb[64:128, lo:lo + 512],
                                         start=True, stop=True)
                        es_e = ep.tile([128, 512], BF16, tag="es", name="ese")
                        nc.scalar.activation(out=es_e, in_=se, func=AF.Exp,
                                             scale=SCALE)
                        if it % 16 == 15:
                            es_o = ep.tile([128, 512], BF16, tag="es",
                                           name="esoa")
                            nc.scalar.activation(out=es_o, in_=so, func=AF.Exp,
                                                 scale=SCALE)
                            es_ob = es_o
                        else:
                            es_o = ep.tile([128, 512], I16, tag="es",
                                           name="esov")
                            nc.vector.tensor_scalar(
                                out=es_o, in0=so, scalar1=EXP_A, scalar2=EXP_B,
                                op0=OP.mult, op1=OP.add)
                            es_ob = es_o.bitcast(BF16)
                        nc.tensor.matmul(ptp_e, v_sb[:, mt, he, :], es_e,
                                         start=(mt == 0), stop=(mt == NMT - 1))
                        nc.tensor.matmul(ptp_o, v_sb[:, mt, ho, :], es_ob,
                                         start=(mt == 0), stop=(mt == NMT - 1))
                        it += 1
                    nc.vector.tensor_copy(out=pt_sb[he][:, lo:lo + 512],
                                          in_=ptp_e)
                    nc.vector.tensor_copy(out=pt_sb[ho][:, lo:lo + 512],
                                          in_=ptp_o)
                    nc.sync.dma_start(out=sums_sb[he:he + 1, lo:lo + 512],
                                      in_=pt_sb[he][C:C + 1, lo:lo + 512])
                    nc.sync.dma_start(out=sums_sb[ho:ho + 1, lo:lo + 512],
                                      in_=pt_sb[ho][C:C + 1, lo:lo + 512])
            for ls in range(NLS):
                phase_d(ls)

    nc.compile()
    return nc


def kernel(x, z, Wk, bk, Wv, bv, Wo, bo):
    x = np.ascontiguousarray(x, dtype=np.float32)
    z = np.ascontiguousarray(z, dtype=np.float32)
    Wk = np.asarray(Wk, np.float32)
    Wv = np.asarray(Wv, np.float32)
    Wo = np.asarray(Wo, np.float32)
    bk = np.asarray(bk, np.float32)
    bv = np.asarray(bv, np.float32)
    bo = np.asarray(bo, np.float32)
    if "nc" not in _CACHE:
        _CACHE["nc"] = build_nc()
    nc = _CACHE["nc"]
    # out-proj is linear, so the constant V bias folds into the residual:
    # ((P + bv*d)/d) @ Wo + bo = (P/d) @ Wo + (bv @ Wo + bo)
    res_bias = bv @ Wo + bo                      # [C]
    shared = {
        "Wk": np.ascontiguousarray(Wk.astype(ml_dtypes.bfloat16)),
        "Wv": np.ascontiguousarray(Wv.astype(ml_dtypes.bfloat16)),
        "Wo": np.ascontiguousarray(
            Wo.reshape(HEADS, C, C).transpose(1, 0, 2)
            .astype(ml_dtypes.bfloat16)),
        "bk": np.ascontiguousarray(bk.reshape(4, 128).T),
        "ones_b": np.ones((128, 128), ml_dtypes.bfloat16),
    }
    in_maps = []
    for core in range(N_CORES):
        bi, half = core // 2, core % 2
        xi = x[bi].reshape(C, L)
        in_maps.append({
            "xq": np.ascontiguousarray(
                xi[:, half * LH:(half + 1) * LH].astype(ml_dtypes.bfloat16)),
            "xr": np.ascontiguousarray(
                x[bi].reshape(-1)[half * LH * C:(half + 1) * LH * C]
                .reshape(NLS, 128, C).transpose(1, 0, 2) + res_bias),
            "zb": np.ascontiguousarray(
                z[bi].reshape(DIM, L).astype(ml_dtypes.bfloat16)),
            **shared,
        })
    _CACHE["in_maps"] = in_maps
    res = run_bass_kernel_spmd(nc, in_maps, list(range(N_CORES)))
    full = np.empty((B, L * C), dtype=np.float32)
    for core in range(N_CORES):
        bi, half = core // 2, core % 2
        full[bi, half * LH * C:(half + 1) * LH * C] = \
            res.results[core]["out"].reshape(-1)
    return full.reshape(B, C, H, W)
